# revision 11
# baseline (speedup 1.0000x reference)
"""MultiHeadAttention (B=2, S=2048, D=1024, H=16, softmax over query axis)
on 8 TRN2 NeuronCores.

Sharding: core c handles batch b = c//4 and head-group hg = c%4 (4 heads,
d_local = 256). QKV weights row-sharded by head group, Wo column-sharded;
each core produces a partial [S, D] output, host sums the 4 partials per
batch and adds the output bias.

Schedule (per core), engineered so PE (164us of matmul) and ACT (128us of
softmax exp) overlap at ~98%:
  startup: Q/K m0 projections as 8 concurrent c-outer PSUM chains that track
    the x DMA stream; evacuated via ACT (bias-add) which is idle here.
  W0 window (wps pair 0 = heads 0,1): scores+exp stream ACT-paced; Q/K m1 +
    V chains fill PE gaps (2-bank pool); exp row-sums via DVE tensor_scalar
    accum (4x mode); AVs deferred (E retained in SBUF) until V + the wps
    PSUM banks are free, then drain interleaved with the remaining scores
    and a prefetch of head-2 scores (keeps ACT busy through the drain).
  W1 window (pair 1 = heads 2,3): prefetched h2 AVs + streaming h3.
  tail: out projection; PSUM->SBUF copies alternate ACT/DVE; DMA per tile.
"""

import os

import numpy as np
import ml_dtypes

import concourse.tile as tile
from concourse import bacc, mybir
from concourse.bass_utils import run_bass_kernel_spmd

B, S, D, H = 2, 2048, 1024, 16
HD = D // H            # 64
NCORES = 8
HPC = H // (NCORES // B)   # heads per core = 4
DL = HPC * HD              # local head dims = 256
CT = D // 128              # 8 contraction tiles over D
ST = S // 128              # 16 seq tiles
BF = mybir.dt.bfloat16
F32 = mybir.dt.float32
bf16 = ml_dtypes.bfloat16
ADD = mybir.AluOpType.add
MUL = mybir.AluOpType.mult

DEBUG_DUMP = False

K1 = 12      # kts of h0/h1 scores emitted inside the chain (qps2) block
N_PRE = 15   # kts of h2 scores prefetched into the W0 drain

_CACHE = {}
LAST_RESULT = None


def _emit_body(nc, tc, aps):
    xT, wqT, wkT, wvT, woT, bq, bk, bv, out = aps
    with tc.tile_pool(name="const", bufs=1) as cp:
        bq_sb = cp.tile([128, 2], F32)
        bk_sb = cp.tile([128, 2], F32)
        bv_row = cp.tile([1, DL], F32)
        bvb = cp.tile([128, DL], F32)

        # Q/K weights come in host-rearranged m-major ([p, m, c, d] order)
        # so each m-half is one contiguous-line DMA; only the m0 halves
        # precede the x stream, the m1 halves follow it
        wq_sb = cp.tile([128, 2 * CT * 128], BF)
        wk_sb = cp.tile([128, 2 * CT * 128], BF)
        nc.sync.dma_start(wq_sb[:, 0:1024], wqT[:, 0:1024])
        nc.sync.dma_start(wk_sb[:, 0:1024], wkT[:, 0:1024])

        def wslice(sb, c, m):
            q0 = m * 1024 + c * 128
            return sb[:, q0:q0 + 128]
        xt_ch = []
        for c in range(CT):
            tx = cp.tile([128, S], BF, tag=f"xt{c}", name=f"xt{c}")
            nc.sync.dma_start(tx[:], xT[c * 128:(c + 1) * 128, :])
            xt_ch.append(tx)
        # biases are only needed by the first evacuations (~16us in)
        nc.sync.dma_start(bq_sb[:], bq)
        nc.sync.dma_start(bk_sb[:], bk)
        nc.sync.dma_start(bv_row[:], bv)
        nc.gpsimd.partition_broadcast(bvb[:], bv_row[:])
        nc.sync.dma_start(wq_sb[:, 1024:2048], wqT[:, 1024:2048])
        nc.sync.dma_start(wk_sb[:, 1024:2048], wkT[:, 1024:2048])
        wv_sb = cp.tile([128, CT, DL], BF)
        nc.sync.dma_start(wv_sb[:], wvT.rearrange("(c p) d -> p c d", p=128))
        wv_c = [wv_sb[:, c, :] for c in range(CT)]
        woT_sb = cp.tile([128, DL // 128, D], BF)
        nc.sync.dma_start(woT_sb[:], woT.rearrange("(c p) j -> p c j", p=128))

        qT_sb = [cp.tile([128, S], BF, tag=f"qT{m}", name=f"qT{m}")
                 for m in range(2)]
        kT_sb = [cp.tile([128, S], BF, tag=f"kT{m}", name=f"kT{m}")
                 for m in range(2)]
        v_sb = cp.tile([128, ST, DL], BF)
        wt_sb = [cp.tile([128, S], BF, tag=f"wt{m}", name=f"wt{m}")
                 for m in range(2)]

        # ---- startup: m0 Q/K chunks 0,1 as 4 c-outer chains tracking the
        # x DMA stream; that is just enough for the q-half-0 scores of
        # kt 0-7, so ACT starts early. Evac via ACT (idle here). The other
        # m0 chunks run later as W0 fillers at full PE clock.
        with tc.tile_pool(name="qk8", bufs=1, space="PSUM") as qk8:
            chains = [(0, 0), (0, 1), (1, 0), (1, 1)]
            pq8 = [qk8.tile([128, 512], F32, tag=f"m0_{i}", name=f"m0_{i}")
                   for i in range(len(chains))]
            for c in range(CT):
                for i, (which, ch) in enumerate(chains):
                    wsb = wq_sb if which == 0 else wk_sb
                    nc.tensor.matmul(
                        pq8[i][:], wslice(wsb, c, 0),
                        xt_ch[c][:, ch * 512:(ch + 1) * 512],
                        start=(c == 0), stop=(c == CT - 1))
            for i, (which, ch) in enumerate(chains):
                dst, bsb = ((qT_sb, bq_sb) if which == 0 else (kT_sb, bk_sb))
                nc.scalar.activation(
                    dst[0][:, ch * 512:(ch + 1) * 512], pq8[i][:],
                    mybir.ActivationFunctionType.Identity, bias=bsb[:, 0:1])

        # ---- attention ----
        def head_slices(h):
            m, off = divmod(h, 2)
            return (qT_sb[m][off * 64:(off + 1) * 64, :],
                    kT_sb[m][off * 64:(off + 1) * 64, :])

        with tc.tile_pool(name="E", bufs=52) as ep, \
             tc.tile_pool(name="ssum", bufs=100) as ssp, \
             tc.tile_pool(name="small", bufs=24) as sp, \
             tc.tile_pool(name="scr", bufs=3) as scr:

            esum = {}   # (h, kt) -> (e0, e1, s0, s1)
            cur_sps = []  # innermost-open scores psum pool

            def sc_unit(h, kt, qh):
                """One [128k x 1024q] scores tile: 2 matmuls + exp + sum."""
                qTh, kTh = head_slices(h)
                sps = cur_sps[-1].tile([128, 1024], F32, tag="sps", name="sps")
                for i in range(2):
                    q0 = qh * 1024 + i * 512
                    nc.tensor.matmul(
                        sps[:, i * 512:(i + 1) * 512],
                        kTh[:, kt * 128:(kt + 1) * 128],
                        qTh[:, q0:q0 + 512], start=True, stop=True)
                e_t = ep.tile([128, 1024], BF, tag="E", name="E")
                nc.scalar.activation(
                    e_t[:], sps[:], mybir.ActivationFunctionType.Exp,
                    scale=float(1.0 / np.sqrt(HD)))
                s_t = ssp.tile([128, 1], F32, tag=f"s{qh}", name=f"s{qh}")
                scr_t = scr.tile([128, 1024], BF, tag="scr", name="scr")
                nc.vector.tensor_scalar(
                    scr_t[:], e_t[:], 1.0, None, MUL, ADD, accum_out=s_t[:])
                lst = esum.setdefault((h, kt), [])
                lst.append((e_t, s_t))

            def vs_of(h, kt):
                (e0, s0), (e1, s1) = esum[(h, kt)]
                stot = sp.tile([128, 1], F32, tag="stot", name="stot")
                nc.vector.tensor_add(stot[:], s0[:], s1[:])
                r = sp.tile([128, 1], F32, tag="r", name="r")
                nc.vector.reciprocal_approx_fast(r[:], stot[:])
                vs = sp.tile([128, HD], BF, tag="vs", name="vs")
                nc.vector.tensor_scalar_mul(
                    vs[:], v_sb[:, kt, (h % HPC) * HD:(h % HPC) * HD + HD], r[:])
                return vs

            def av(wps, h, kt, first=None, last=None):
                vs = vs_of(h, kt)
                (e0, _), (e1, _) = esum.pop((h, kt))
                off = 64 * (h % 2)
                tp = (0, off) if off else None
                if first is None:
                    first = kt == 0
                if last is None:
                    last = kt == ST - 1
                for qh, e_t in ((0, e0), (1, e1)):
                    for i in range(2):
                        q0 = qh * 1024 + i * 512
                        nc.tensor.matmul(
                            wps[off:off + 64, q0:q0 + 512], vs[:],
                            e_t[:, i * 512:(i + 1) * 512],
                            start=first, stop=last,
                            tile_position=tp)

            # --- W0 part 1: scores h0/h1 kt<K1, m1 + V chains as PE filler ---
            # filler thunk streams: m1 first (needed by the h2 prefetch), V
            # second (needed when the AV drain starts).
            filler = []
            with tc.tile_pool(name="sps3", bufs=3, space="PSUM") as sps3, \
                 tc.tile_pool(name="mv_ps", bufs=2, space="PSUM") as qps2:
                cur_sps.append(sps3)
                def qk_chain(which, m, ch, act_evac=False):
                    wsb, bsb, dst = ((wq_sb, bq_sb, qT_sb) if which == 0
                                     else (wk_sb, bk_sb, kT_sb))
                    pq = qps2.tile([128, 512], F32, tag="mv", name="mv")
                    def step(c, pq=pq, wsb=wsb, m=m, ch=ch):
                        nc.tensor.matmul(
                            pq[:], wslice(wsb, c, m),
                            xt_ch[c][:, ch * 512:(ch + 1) * 512],
                            start=(c == 0), stop=(c == CT - 1))
                    def evac(pq=pq, dst=dst, bsb=bsb, m=m, ch=ch):
                        if act_evac:
                            # keeps the unblock off the TS-congested DVE queue
                            nc.scalar.activation(
                                dst[m][:, ch * 512:(ch + 1) * 512], pq[:],
                                mybir.ActivationFunctionType.Identity,
                                bias=bsb[:, m:m + 1])
                        else:
                            nc.vector.tensor_scalar_add(
                                dst[m][:, ch * 512:(ch + 1) * 512], pq[:],
                                bsb[:, m:m + 1])
                    return [lambda c=c: step(c) for c in range(CT)] + [evac]

                def v_chain(st, act_evac=False):
                    # one V st-chain per full-bank tile: a PSUM zero-region
                    # is a whole bank, so concurrent chains can't share one
                    pv = qps2.tile([128, 512], F32, tag="mv", name="mv")
                    steps = []
                    for c in range(CT):
                        def step(c=c, pv=pv, st=st):
                            nc.tensor.matmul(
                                pv[:, 0:DL],
                                xt_ch[c][:, st * 128:(st + 1) * 128],
                                wv_c[c][:],
                                start=(c == 0), stop=(c == CT - 1))
                        steps.append(step)
                    def evac(pv=pv, st=st):
                        if act_evac:
                            # last chains: unblock the wps banks without
                            # sitting behind exp-gated sums in the DVE queue;
                            # bias added afterwards on gpsimd (SBUF-only)
                            nc.scalar.activation(
                                v_sb[:, st, :], pv[:, 0:DL],
                                mybir.ActivationFunctionType.Copy)
                            nc.gpsimd.tensor_add(
                                v_sb[:, st, :], v_sb[:, st, :], bvb[:])
                        else:
                            nc.vector.tensor_add(
                                v_sb[:, st, :], pv[:, 0:DL], bvb[:])
                    return steps + [evac]

                # m0 chunks 2,3 first (unblocks q-half-1 and kt>=8 scores),
                # then m1 (unblocks the h2 prefetch), then V (needed once
                # the AV drain starts).
                costs = []
                for which in range(2):
                    for ch in (2, 3):
                        t = qk_chain(which, 0, ch)
                        filler.extend(t)
                        costs.extend([512] * CT + [0])
                for which in range(2):
                    for ch in range(4):
                        t = qk_chain(which, 1, ch)
                        filler.extend(t)
                        costs.extend([512] * CT + [0])
                for st in range(ST):
                    t = v_chain(st)
                    filler.extend(t)
                    costs.extend([256] * CT + [0])
                cum = []
                run = 0
                for cst in costs:
                    run += cst
                    cum.append(run)
                total_rows = run

                # unit order: q-half-0 of kt 0-7 first (only those are
                # unblocked by the startup chains), then the rest
                units = [(h, kt, 0) for kt in range(8) for h in (0, 1)]
                units += [(h, kt, 1) for kt in range(8) for h in (0, 1)]
                units += [(h, kt, qh) for kt in range(8, K1)
                          for h in (0, 1) for qh in range(2)]
                fi = 0
                for done, (h, kt, qh) in enumerate(units):
                    sc_unit(h, kt, qh)
                    tgt_rows = total_rows * (done + 1) // len(units)
                    while fi < len(filler) and cum[fi] <= tgt_rows:
                        filler[fi]()
                        fi += 1
                while fi < len(filler):
                    filler[fi]()
                    fi += 1
                cur_sps.pop()

            # --- W0 part 2: drain AVs of h0/h1, finish their scores, and
            # prefetch h2 scores to keep ACT fed ---
            with tc.tile_pool(name="sps2", bufs=2, space="PSUM") as sps2, \
                 tc.tile_pool(name="wps", bufs=1, space="PSUM") as wpsp:
                cur_sps.append(sps2)
                wps0 = wpsp.tile([128, S], F32, tag="wps", name="wps0")
                pre = 0
                for kt in range(ST + 1):
                    if K1 <= kt < ST:
                        for h in (0, 1):
                            for qh in range(2):
                                sc_unit(h, kt, qh)
                    # h2 prefetch front-loaded: drain kts < K1 have no
                    # h0/h1 score units, so ACT feeds on h2 units there
                    tgt = min(N_PRE, ((kt + 1) * N_PRE + K1 - 1) // K1)
                    while pre < tgt:
                        for qh in range(2):
                            sc_unit(2, pre, qh)
                        pre += 1
                    # AVs lag the score stream by one kt so PE never parks
                    # on the exp->sum->vs chain of the kt it just emitted
                    if kt >= 1:
                        av(wps0, 0, kt - 1)
                        av(wps0, 1, kt - 1)
                # wt0 evac split ACT||DVE: halves the wps1-allocation gate
                nc.scalar.copy(wt_sb[0][:, 0:1024], wps0[:, 0:1024])
                nc.vector.tensor_copy(wt_sb[0][:, 1024:2048],
                                      wps0[:, 1024:2048])

                # --- W1: pair 1 (h2, h3) ---
                # kt order [15, 0..14]: the final AVs then depend on exps
                # computed two iterations back, not on the freshest ones
                wps1 = wpsp.tile([128, S], F32, tag="wps", name="wps1")
                kts = [ST - 1] + list(range(ST - 1))
                for j in range(len(kts) + 1):
                    if j < len(kts):
                        kt = kts[j]
                        if kt >= N_PRE:
                            for qh in range(2):
                                sc_unit(2, kt, qh)
                        for qh in range(2):
                            sc_unit(3, kt, qh)
                    if j >= 1:
                        av(wps1, 2, kts[j - 1], first=(j == 1),
                           last=(j == len(kts)))
                        av(wps1, 3, kts[j - 1], first=(j == 1),
                           last=(j == len(kts)))
                # wt1 evac split across ACT (idle after the last exp) and
                # DVE in parallel; chunk 0 first, it gates out-proj st 0-7
                nc.scalar.copy(wt_sb[1][:, 0:1024], wps1[:, 0:1024])
                nc.vector.tensor_copy(wt_sb[1][:, 1024:2048],
                                      wps1[:, 1024:2048])

        if DEBUG_DUMP:
            dbg = _CACHE["dbg_aps"]
            for m in range(2):
                nc.sync.dma_start(dbg["dq"][m], qT_sb[m][:])
                nc.sync.dma_start(dbg["dk"][m], kT_sb[m][:])
                nc.sync.dma_start(dbg["dwt"][m], wt_sb[m][:])
            nc.sync.dma_start(dbg["dv"], v_sb[:])

        # ---- output projection (partial over local heads) ----
        with tc.tile_pool(name="out_sb", bufs=6) as osb, \
             tc.tile_pool(name="ops", bufs=4, space="PSUM") as ops:
            for st in range(ST):
                po = ops.tile([128, D], F32, tag="po", name="po")
                for ch in range(2):
                    for c in range(2):
                        nc.tensor.matmul(
                            po[:, ch * 512:(ch + 1) * 512],
                            wt_sb[c][:, st * 128:(st + 1) * 128],
                            woT_sb[:, c, ch * 512:(ch + 1) * 512],
                            start=(c == 0), stop=(c == 1))
                ob = osb.tile([128, D], BF, tag="ob", name="ob")
                if st % 2 == 0:
                    nc.scalar.copy(ob[:], po[:])
                else:
                    nc.vector.tensor_copy(ob[:], po[:])
                nc.sync.dma_start(out[st * 128:(st + 1) * 128, :], ob[:])


def _build(reps=None, marker=False):
    """reps=None: single-shot kernel. reps=N: python-unrolled N repetitions
    of the whole body (benchmarking only). marker adds a dummy input named
    by reps so differently-unrolled builds can't alias in any compile cache."""
    nc = bacc.Bacc("TRN2", target_bir_lowering=False, debug=False,
                   num_devices=NCORES)
    if marker:
        nc.dram_tensor(f"repmark{reps or 1}", [1, 1], F32,
                       kind="ExternalInput")
    xT = nc.dram_tensor("xT", [D, S], BF, kind="ExternalInput").ap()
    wqT = nc.dram_tensor("wqT", [128, 2 * CT * 128], BF, kind="ExternalInput").ap()
    wkT = nc.dram_tensor("wkT", [128, 2 * CT * 128], BF, kind="ExternalInput").ap()
    wvT = nc.dram_tensor("wvT", [D, DL], BF, kind="ExternalInput").ap()
    woT = nc.dram_tensor("woT", [DL, D], BF, kind="ExternalInput").ap()
    bq = nc.dram_tensor("bq", [128, 2], F32, kind="ExternalInput").ap()
    bk = nc.dram_tensor("bk", [128, 2], F32, kind="ExternalInput").ap()
    bv = nc.dram_tensor("bv", [1, DL], F32, kind="ExternalInput").ap()
    out = nc.dram_tensor("out", [S, D], BF, kind="ExternalOutput").ap()
    if DEBUG_DUMP:
        _CACHE["dbg_aps"] = {
            "dq": nc.dram_tensor("dq", [2, 128, S], BF, kind="ExternalOutput").ap(),
            "dk": nc.dram_tensor("dk", [2, 128, S], BF, kind="ExternalOutput").ap(),
            "dwt": nc.dram_tensor("dwt", [2, 128, S], BF, kind="ExternalOutput").ap(),
            "dv": nc.dram_tensor("dv", [128, ST, DL], BF, kind="ExternalOutput").ap(),
        }
    aps = (xT, wqT, wkT, wvT, woT, bq, bk, bv, out)

    with tile.TileContext(nc) as tc:
        for _ in range(reps or 1):
            _emit_body(nc, tc, aps)

    nc.compile()
    return nc


def _get_nc():
    if "nc" not in _CACHE:
        _CACHE["nc"] = _build()
    return _CACHE["nc"]


def _make_in_maps(x, wq, bq, wk, bk, wv, bv, wo):
    xTs = [np.ascontiguousarray(x[b].T).astype(bf16) for b in range(B)]
    in_maps = []
    for core in range(NCORES):
        b, hg = core // (NCORES // B), core % (NCORES // B)
        rows = slice(hg * DL, (hg + 1) * DL)
        in_maps.append({
            "xT": xTs[b],
            "wqT": np.ascontiguousarray(
                wq[rows].T.reshape(8, 128, 2, 128).transpose(1, 2, 0, 3)
                .reshape(128, 2048)).astype(bf16),
            "wkT": np.ascontiguousarray(
                wk[rows].T.reshape(8, 128, 2, 128).transpose(1, 2, 0, 3)
                .reshape(128, 2048)).astype(bf16),
            "wvT": np.ascontiguousarray(wv[rows].T).astype(bf16),
            "woT": np.ascontiguousarray(wo[:, rows].T).astype(bf16),
            "bq": np.ascontiguousarray(bq[rows].reshape(2, 128).T),
            "bk": np.ascontiguousarray(bk[rows].reshape(2, 128).T),
            "bv": np.ascontiguousarray(bv[rows].reshape(1, DL)),
        })
    return in_maps


def kernel(x, wq, bq, wk, bk, wv, bv, wo, bo):
    global LAST_RESULT
    x = np.asarray(x, dtype=np.float32)
    wq, bq = np.asarray(wq, np.float32), np.asarray(bq, np.float32)
    wk, bk = np.asarray(wk, np.float32), np.asarray(bk, np.float32)
    wv, bv = np.asarray(wv, np.float32), np.asarray(bv, np.float32)
    wo, bo = np.asarray(wo, np.float32), np.asarray(bo, np.float32)

    nc = _get_nc()
    in_maps = _make_in_maps(x, wq, bq, wk, bk, wv, bv, wo)

    trace = os.environ.get("MHA_TRACE", "0") == "1"
    res = run_bass_kernel_spmd(nc, in_maps, core_ids=list(range(NCORES)),
                               trace=trace)
    LAST_RESULT = res

    out = np.zeros((B, S, D), np.float32)
    for core in range(NCORES):
        out[core // (NCORES // B)] += res.results[core]["out"].astype(np.float32)
    out += bo[None, None, :]
    return out
